# revision 2
# baseline (speedup 1.0000x reference)
"""AKDN GNN message-passing kernel for 8 TRN2 NeuronCores (Bass SPMD).

Edges/nnz are destination-sharded across 8 cores. Each core's NEFF computes,
per layer: per-edge attention logits (dot of gathered tail rows with
relation-projected weights), leaky-relu + exp softmax weights, weighted
payloads, and the two segment-sum aggregations (KG attention aggregation and
the interaction-graph SpMM) via layered unique-destination dma_scatter_add
rounds (CCE accumulate in the DMA datapath). The same compiled graph is
invoked once per layer; the host re-stages per-edge rows between layers and
applies the cheap fusion gate / final scoring matmul.
"""
import sys
sys.path.insert(0, "/opt/trn_rl_repo")
sys.path.insert(0, "/root/.axon_site")
import numpy as np

N_ENT = 100000
N_USR = 30000
N_TOT = N_ENT + N_USR
D = 64
P = 128
SLOPE = 0.01
NCORE = 8
EK_SH = 12500          # KG dest rows per core
EI_SH_I = 12500        # IG item dest rows per core
EI_SH_U = 3750         # IG user dest rows per core
ACC_K = 12544          # KG acc rows (12500 + trash + pad)
ACC_I = 16384          # IG acc rows (12500 item + 3750 user + trash + pad)
TRASH_K = 12500
IG_UOFF = 12544        # local offset of user rows in IG acc
TRASH_I = 16300
MAXCAP = 3840          # max scatter descs per instruction (mult of 128)

LAST_EXEC_NS = []


def _wrap16(idx, n_slots):
    a = np.zeros(n_slots, dtype=np.int16)
    a[: len(idx)] = idx
    a = a.reshape(-1, 16).T
    return np.tile(a, (8, 1)).copy()


def _rounds(dest):
    """Split edge list into rounds with unique destinations.
    Returns list of arrays of edge ids."""
    order = np.argsort(dest, kind="stable")
    sd = dest[order]
    n = len(sd)
    if n == 0:
        return []
    first = np.r_[True, sd[1:] != sd[:-1]]
    seg_id = np.cumsum(first) - 1
    seg_start = np.flatnonzero(first)
    pos = np.arange(n) - seg_start[seg_id]
    out = []
    for j in range(int(pos.max()) + 1):
        idx = order[pos == j]
        for s in range(0, len(idx), MAXCAP):
            out.append(idx[s : s + MAXCAP])
    return out


def _build_graph(ekp, eip, kg_sizes, ig_sizes):
    import concourse.bass as bass
    import concourse.tile as tile
    from concourse import bacc, mybir

    f32 = mybir.dt.float32
    i16 = mybir.dt.int16
    nc = bacc.Bacc("TRN2", target_bir_lowering=False, debug=False)

    kg_T = nc.declare_dram_parameter("kg_T", [ekp, D], f32, isOutput=False)
    kg_A = nc.declare_dram_parameter("kg_A", [ekp, D], f32, isOutput=False)
    kg_q = nc.declare_dram_parameter("kg_q", [P, ekp // P], f32, isOutput=False)
    kg_s = nc.declare_dram_parameter("kg_s", [P, ekp // 16], i16, isOutput=False)
    ig_R = nc.declare_dram_parameter("ig_R", [eip, D], f32, isOutput=False)
    ig_v = nc.declare_dram_parameter("ig_v", [P, eip // P], f32, isOutput=False)
    ig_s = nc.declare_dram_parameter("ig_s", [P, eip // 16], i16, isOutput=False)
    acc_k = nc.declare_dram_parameter("acc_k", [ACC_K, 2 * D], f32, isOutput=True)
    acc_i = nc.declare_dram_parameter("acc_i", [ACC_I, D], f32, isOutput=True)

    with tile.TileContext(nc) as tc:
        with tc.tile_pool(name="ip", bufs=1) as ip, tc.tile_pool(name="sb", bufs=2) as sb:
            ks_t = ip.tile([P, ekp // 16], i16)
            is_t = ip.tile([P, eip // 16], i16)
            nc.sync.dma_start(out=ks_t[:], in_=kg_s[:, :])
            nc.sync.dma_start(out=is_t[:], in_=ig_s[:, :])

            s0 = 0
            for n_r in kg_sizes:
                g = n_r // P
                T_t = sb.tile([P, g, D], f32)
                A_t = sb.tile([P, g, D], f32)
                q_t = sb.tile([P, g], f32)
                nc.sync.dma_start(
                    out=T_t[:], in_=kg_T[s0 : s0 + n_r, :].rearrange("(g p) d -> p g d", p=P)
                )
                nc.sync.dma_start(
                    out=A_t[:], in_=kg_A[s0 : s0 + n_r, :].rearrange("(g p) d -> p g d", p=P)
                )
                nc.sync.dma_start(out=q_t[:], in_=kg_q[:, s0 // P : s0 // P + g])
                prod = sb.tile([P, g, D], f32)
                nc.vector.tensor_tensor(out=prod[:], in0=T_t[:], in1=A_t[:], op=mybir.AluOpType.mult)
                v_t = sb.tile([P, g], f32)
                nc.vector.tensor_reduce(out=v_t[:], in_=prod[:], axis=mybir.AxisListType.X, op=mybir.AluOpType.add)
                nc.vector.tensor_tensor(out=v_t[:], in0=v_t[:], in1=q_t[:], op=mybir.AluOpType.add)
                w_t = sb.tile([P, g], f32)
                nc.scalar.activation(out=w_t[:], in_=v_t[:], func=mybir.ActivationFunctionType.Lrelu, alpha=SLOPE)
                nc.scalar.activation(out=w_t[:], in_=w_t[:], func=mybir.ActivationFunctionType.Exp)
                pay = sb.tile([P, g, 2 * D], f32)
                nc.vector.tensor_tensor(
                    out=pay[:, :, 0:D],
                    in0=T_t[:],
                    in1=w_t[:, :, None].to_broadcast([P, g, D]),
                    op=mybir.AluOpType.mult,
                )
                nc.vector.tensor_copy(out=pay[:, :, D : D + 1], in_=w_t[:, :, None])
                nc.vector.memset(pay[:, :, D + 1 :], 0.0)
                nc.gpsimd.dma_scatter_add(
                    acc_k[:, :], pay[:], ks_t[:, s0 // 16 : (s0 + n_r) // 16],
                    n_r, n_r, 2 * D, single_packet=False,
                )
                s0 += n_r

            s0 = 0
            for n_r in ig_sizes:
                g = n_r // P
                R_t = sb.tile([P, g, D], f32)
                v_t = sb.tile([P, g], f32)
                nc.sync.dma_start(
                    out=R_t[:], in_=ig_R[s0 : s0 + n_r, :].rearrange("(g p) d -> p g d", p=P)
                )
                nc.sync.dma_start(out=v_t[:], in_=ig_v[:, s0 // P : s0 // P + g])
                pay = sb.tile([P, g, D], f32)
                nc.vector.tensor_tensor(
                    out=pay[:],
                    in0=R_t[:],
                    in1=v_t[:, :, None].to_broadcast([P, g, D]),
                    op=mybir.AluOpType.mult,
                )
                nc.gpsimd.dma_scatter_add(
                    acc_i[:, :], pay[:], is_t[:, s0 // 16 : (s0 + n_r) // 16],
                    n_r, n_r, D, single_packet=False,
                )
                s0 += n_r
    nc.compile()
    return nc


def kernel(all_embed, rel_embed, Wk_w, Wk_b, Wa_w, Wb_w, a_vals,
           user_ids, item_ids, h_list, t_list, r_list, a_row, a_col):
    from concourse.bass_utils import run_bass_kernel_spmd

    global LAST_EXEC_NS
    LAST_EXEC_NS = []
    f = np.float32
    all_embed = np.asarray(all_embed, f)
    rel_embed = np.asarray(rel_embed, f)
    Wk_w = np.asarray(Wk_w, f)
    Wk_b = np.asarray(Wk_b, f)
    Wa_w = np.asarray(Wa_w, f)
    Wb_w = np.asarray(Wb_w, f)
    a_vals = np.asarray(a_vals, f)
    user_ids = np.asarray(user_ids).astype(np.int64)
    item_ids = np.asarray(item_ids).astype(np.int64)
    h_list = np.asarray(h_list).astype(np.int64)
    t_list = np.asarray(t_list).astype(np.int64)
    r_list = np.asarray(r_list).astype(np.int64)
    a_row = np.asarray(a_row).astype(np.int64)
    a_col = np.asarray(a_col).astype(np.int64)

    AB = rel_embed @ Wk_w          # (32, 128)
    A_tab = AB[:, :D]              # tail-side projection per relation
    B_tab = AB[:, D:]              # head-side projection per relation
    c_tab = rel_embed @ Wk_b       # (32,)

    # ---- per-core edge assignment (destination sharding) ----
    kg_core = np.minimum(h_list // EK_SH, NCORE - 1)
    ig_part_item = a_row < N_ENT
    ig_core = np.where(ig_part_item,
                       np.minimum(a_row // EI_SH_I, NCORE - 1),
                       np.minimum((a_row - N_ENT) // EI_SH_U, NCORE - 1))
    ig_local = np.where(ig_part_item,
                        a_row - (np.minimum(a_row // EI_SH_I, NCORE - 1)) * EI_SH_I,
                        IG_UOFF + (a_row - N_ENT) - (np.minimum((a_row - N_ENT) // EI_SH_U, NCORE - 1)) * EI_SH_U)
    kg_local = h_list - kg_core * EK_SH

    # ---- rounds per core (shared across both layers: same index data) ----
    kg_rounds = [_rounds(kg_local[kg_core == c]) for c in range(NCORE)]
    ig_rounds = [_rounds(ig_local[ig_core == c]) for c in range(NCORE)]
    kg_eids = [np.flatnonzero(kg_core == c) for c in range(NCORE)]
    ig_eids = [np.flatnonzero(ig_core == c) for c in range(NCORE)]

    nrk = max(len(r) for r in kg_rounds)
    nri = max(len(r) for r in ig_rounds)
    kg_sizes = [max((len(kg_rounds[c][j]) if j < len(kg_rounds[c]) else 1) for c in range(NCORE)) for j in range(nrk)]
    ig_sizes = [max((len(ig_rounds[c][j]) if j < len(ig_rounds[c]) else 1) for c in range(NCORE)) for j in range(nri)]
    kg_sizes = [((s + P - 1) // P) * P for s in kg_sizes]
    ig_sizes = [((s + P - 1) // P) * P for s in ig_sizes]
    ekp = sum(kg_sizes)
    eip = sum(ig_sizes)

    # per-core slot-ordered edge arrays
    kg_slots = []   # (t_idx, r_idx, h_local, valid) per core in slot order
    ig_slots = []
    for c in range(NCORE):
        tks, rks, sks, val = [], [], [], []
        for j, cap in enumerate(kg_sizes):
            if j < len(kg_rounds[c]):
                e = kg_eids[c][kg_rounds[c][j]]
            else:
                e = np.empty(0, np.int64)
            pad = cap - len(e)
            tks.append(np.r_[t_list[e], np.zeros(pad, np.int64)])
            rks.append(np.r_[r_list[e], np.zeros(pad, np.int64)])
            sks.append(np.r_[kg_local[e], np.full(pad, TRASH_K, np.int64)])
            val.append(np.r_[np.ones(len(e), bool), np.zeros(pad, bool)])
        kg_slots.append((np.concatenate(tks), np.concatenate(rks),
                         np.concatenate(sks), np.concatenate(val)))
        cks, vks, sks2 = [], [], []
        for j, cap in enumerate(ig_sizes):
            if j < len(ig_rounds[c]):
                e = ig_eids[c][ig_rounds[c][j]]
            else:
                e = np.empty(0, np.int64)
            pad = cap - len(e)
            cks.append(np.r_[a_col[e], np.zeros(pad, np.int64)])
            vks.append(np.r_[a_vals[e], np.zeros(pad, f)])
            sks2.append(np.r_[ig_local[e], np.full(pad, TRASH_I, np.int64)])
        ig_slots.append((np.concatenate(cks), np.concatenate(vks),
                         np.concatenate(sks2)))

    nc = _build_graph(ekp, eip, kg_sizes, ig_sizes)

    def slotview(x):
        # slot i lives at [i%128, i//128] on device
        return np.ascontiguousarray(x.reshape(-1, P).T)

    def run_layer(e_ent_curr, ig_in):
        q2_all = e_ent_curr @ B_tab.T + c_tab[None, :]   # (N_ENT, 32)
        in_maps = []
        for c in range(NCORE):
            tk, rk, sk, val = kg_slots[c]
            T = e_ent_curr[tk]
            A = A_tab[rk] * val[:, None]
            q = np.where(val, q2_all[np.minimum(sk + c * EK_SH, N_ENT - 1), rk], -1e4).astype(f)
            ck, vv, si = ig_slots[c]
            R = ig_in[ck]
            in_maps.append(dict(
                kg_T=T.astype(f), kg_A=A.astype(f), kg_q=slotview(q),
                kg_s=_wrap16(sk.astype(np.int16), ekp),
                ig_R=R.astype(f), ig_v=slotview(vv.astype(f)),
                ig_s=_wrap16(si.astype(np.int16), eip),
            ))
        res = run_bass_kernel_spmd(nc, in_maps, list(range(NCORE)))
        if res.exec_time_ns:
            LAST_EXEC_NS.append(res.exec_time_ns)
        kg_full = np.empty((N_ENT, D), f)
        ig_full = np.empty((N_TOT, D), f)
        for c in range(NCORE):
            ak = np.asarray(res.results[c]["acc_k"], f)
            ai = np.asarray(res.results[c]["acc_i"], f)
            num = ak[:EK_SH, :D]
            den = ak[:EK_SH, D : D + 1]
            kg_full[c * EK_SH : (c + 1) * EK_SH] = num / (den + 1e-20)
            ig_full[c * EI_SH_I : (c + 1) * EI_SH_I] = ai[:EI_SH_I, :]
            ig_full[N_ENT + c * EI_SH_U : N_ENT + (c + 1) * EI_SH_U] = ai[IG_UOFF : IG_UOFF + EI_SH_U, :]
        return kg_full, ig_full

    e_ent = all_embed[:N_ENT]
    e_usr = all_embed[N_ENT:]
    e_ent_curr, e_dual, e_users = e_ent, e_ent, e_usr
    item_sum = e_ent.copy()
    user_sum = e_usr.copy()
    for _ in range(2):
        kg, ig = run_layer(e_ent_curr, np.concatenate([e_dual, e_users], 0))
        collab = ig[:N_ENT]
        users_new = ig[N_ENT:]
        gate = 1.0 / (1.0 + np.exp(-(kg @ Wa_w.T + collab @ Wb_w.T)))
        e_dual = gate * kg + (1.0 - gate) * collab
        item_sum += collab
        user_sum += users_new
        e_users = users_new
        e_ent_curr = kg

    all_final = np.concatenate([item_sum, user_sum], 0)
    return (all_final[user_ids] @ all_final[item_ids].T).astype(f)


# revision 5
# speedup vs baseline: 1.4458x; 1.4458x over previous
"""AKDN GNN message-passing kernel for 8 TRN2 NeuronCores (Bass SPMD).

Edges/nnz are destination-sharded across 8 cores. Each core's NEFF computes,
per layer: per-edge attention logits (dot of gathered tail rows with
relation-projected weights), leaky-relu + exp softmax weights, weighted
payloads, and the two segment-sum aggregations (KG attention aggregation and
the interaction-graph SpMM) via layered unique-destination dma_scatter_add
rounds (CCE accumulate in the DMA datapath). The same compiled graph is
invoked once per layer; the host re-stages per-edge rows between layers and
applies the cheap fusion gate / final scoring matmul.
"""
import sys
sys.path.insert(0, "/opt/trn_rl_repo")
sys.path.insert(0, "/root/.axon_site")
import numpy as np

N_ENT = 100000
N_USR = 30000
N_TOT = N_ENT + N_USR
D = 64
P = 128
SLOPE = 0.01
NCORE = 8
EK_SH = 12500          # KG dest rows per core
EI_SH_I = 12500        # IG item dest rows per core
EI_SH_U = 3750         # IG user dest rows per core
ACC_K = 12544          # KG acc rows (12500 + trash + pad)
ACC_I = 16384          # IG acc rows (12500 item + 3750 user + trash + pad)
TRASH_K = 12500
IG_UOFF = 12544        # local offset of user rows in IG acc
TRASH_I = 16300
MAXCAP = 3840          # max scatter descs per instruction (mult of 128)

LAST_EXEC_NS = []


def _wrap16(idx, n_slots):
    a = np.zeros(n_slots, dtype=np.int16)
    a[: len(idx)] = idx
    a = a.reshape(-1, 16).T
    return np.tile(a, (8, 1)).copy()


def _rounds(dest):
    """Split edge list into rounds with unique destinations.
    Returns list of arrays of edge ids."""
    order = np.argsort(dest, kind="stable")
    sd = dest[order]
    n = len(sd)
    if n == 0:
        return []
    first = np.r_[True, sd[1:] != sd[:-1]]
    seg_id = np.cumsum(first) - 1
    seg_start = np.flatnonzero(first)
    pos = np.arange(n) - seg_start[seg_id]
    out = []
    for j in range(int(pos.max()) + 1):
        idx = order[pos == j]
        for s in range(0, len(idx), MAXCAP):
            out.append(idx[s : s + MAXCAP])
    return out


def _build_graph(ekp, eip, kg_sizes, ig_sizes):
    import concourse.bass as bass
    import concourse.tile as tile
    from concourse import bacc, mybir

    f32 = mybir.dt.float32
    i16 = mybir.dt.int16
    nc = bacc.Bacc("TRN2", target_bir_lowering=False, debug=False)

    kg_T = nc.declare_dram_parameter("kg_T", [ekp, D], f32, isOutput=False)
    kg_A = nc.declare_dram_parameter("kg_A", [ekp, D], f32, isOutput=False)
    kg_q = nc.declare_dram_parameter("kg_q", [P, ekp // P], f32, isOutput=False)
    kg_s = nc.declare_dram_parameter("kg_s", [P, ekp // 16], i16, isOutput=False)
    ig_R = nc.declare_dram_parameter("ig_R", [eip, D], f32, isOutput=False)
    ig_v = nc.declare_dram_parameter("ig_v", [P, eip // P], f32, isOutput=False)
    ig_s = nc.declare_dram_parameter("ig_s", [P, eip // 16], i16, isOutput=False)
    acc_k = nc.declare_dram_parameter("acc_k", [ACC_K, 2 * D], f32, isOutput=True)
    acc_i = nc.declare_dram_parameter("acc_i", [ACC_I, D], f32, isOutput=True)

    with tile.TileContext(nc) as tc:
        with tc.tile_pool(name="ip", bufs=1) as ip, tc.tile_pool(name="sb", bufs=2) as sb:
            ks_t = ip.tile([P, ekp // 16], i16)
            is_t = ip.tile([P, eip // 16], i16)
            nc.sync.dma_start(out=ks_t[:], in_=kg_s[:, :])
            nc.sync.dma_start(out=is_t[:], in_=ig_s[:, :])

            def emit_kg(s0, n_r):
                g = n_r // P
                T_t = sb.tile([P, g, D], f32)
                A_t = sb.tile([P, g, D], f32)
                q_t = sb.tile([P, g], f32)
                nc.sync.dma_start(
                    out=T_t[:], in_=kg_T[s0 : s0 + n_r, :].rearrange("(g p) d -> p g d", p=P)
                )
                nc.sync.dma_start(
                    out=A_t[:], in_=kg_A[s0 : s0 + n_r, :].rearrange("(g p) d -> p g d", p=P)
                )
                nc.sync.dma_start(out=q_t[:], in_=kg_q[:, s0 // P : s0 // P + g])
                prod = sb.tile([P, g, D], f32)
                nc.vector.tensor_tensor(out=prod[:], in0=T_t[:], in1=A_t[:], op=mybir.AluOpType.mult)
                v_t = sb.tile([P, g], f32)
                nc.vector.tensor_reduce(out=v_t[:], in_=prod[:], axis=mybir.AxisListType.X, op=mybir.AluOpType.add)
                nc.vector.tensor_tensor(out=v_t[:], in0=v_t[:], in1=q_t[:], op=mybir.AluOpType.add)
                w_t = sb.tile([P, g], f32)
                nc.scalar.activation(out=w_t[:], in_=v_t[:], func=mybir.ActivationFunctionType.Lrelu, alpha=SLOPE)
                nc.scalar.activation(out=w_t[:], in_=w_t[:], func=mybir.ActivationFunctionType.Exp)
                pay = sb.tile([P, g, 2 * D], f32)
                nc.vector.tensor_tensor(
                    out=pay[:, :, 0:D],
                    in0=T_t[:],
                    in1=w_t[:, :, None].to_broadcast([P, g, D]),
                    op=mybir.AluOpType.mult,
                )
                nc.vector.tensor_copy(out=pay[:, :, D : D + 1], in_=w_t[:, :, None])
                nc.vector.memset(pay[:, :, D + 1 :], 0.0)
                nc.gpsimd.dma_scatter_add(
                    acc_k[:, :], pay[:], ks_t[:, s0 // 16 : (s0 + n_r) // 16],
                    n_r, n_r, 2 * D, single_packet=False,
                )

            def emit_ig(s0, n_r):
                g = n_r // P
                R_t = sb.tile([P, g, D], f32)
                v_t = sb.tile([P, g], f32)
                nc.sync.dma_start(
                    out=R_t[:], in_=ig_R[s0 : s0 + n_r, :].rearrange("(g p) d -> p g d", p=P)
                )
                nc.sync.dma_start(out=v_t[:], in_=ig_v[:, s0 // P : s0 // P + g])
                pay = sb.tile([P, g, D], f32)
                nc.vector.tensor_tensor(
                    out=pay[:],
                    in0=R_t[:],
                    in1=v_t[:, :, None].to_broadcast([P, g, D]),
                    op=mybir.AluOpType.mult,
                )
                nc.gpsimd.dma_scatter_add(
                    acc_i[:, :], pay[:], is_t[:, s0 // 16 : (s0 + n_r) // 16],
                    n_r, n_r, D, single_packet=False,
                )

            # interleave KG and IG rounds: they scatter into different
            # accumulators, so the Q7 desc-gen of one fills the drain-wait
            # gaps of the other.
            k_offs, o = [], 0
            for n_r in kg_sizes:
                k_offs.append((o, n_r))
                o += n_r
            i_offs, o = [], 0
            for n_r in ig_sizes:
                i_offs.append((o, n_r))
                o += n_r
            nk, ni = len(k_offs), len(i_offs)
            for j in range(max(nk, ni)):
                if j < nk:
                    emit_kg(*k_offs[j])
                if j < ni:
                    emit_ig(*i_offs[j])
    nc.compile()
    return nc


def kernel(all_embed, rel_embed, Wk_w, Wk_b, Wa_w, Wb_w, a_vals,
           user_ids, item_ids, h_list, t_list, r_list, a_row, a_col):
    from concourse.bass_utils import run_bass_kernel_spmd

    global LAST_EXEC_NS
    LAST_EXEC_NS = []
    f = np.float32
    all_embed = np.asarray(all_embed, f)
    rel_embed = np.asarray(rel_embed, f)
    Wk_w = np.asarray(Wk_w, f)
    Wk_b = np.asarray(Wk_b, f)
    Wa_w = np.asarray(Wa_w, f)
    Wb_w = np.asarray(Wb_w, f)
    a_vals = np.asarray(a_vals, f)
    user_ids = np.asarray(user_ids).astype(np.int64)
    item_ids = np.asarray(item_ids).astype(np.int64)
    h_list = np.asarray(h_list).astype(np.int64)
    t_list = np.asarray(t_list).astype(np.int64)
    r_list = np.asarray(r_list).astype(np.int64)
    a_row = np.asarray(a_row).astype(np.int64)
    a_col = np.asarray(a_col).astype(np.int64)

    AB = rel_embed @ Wk_w          # (32, 128)
    A_tab = AB[:, :D]              # tail-side projection per relation
    B_tab = AB[:, D:]              # head-side projection per relation
    c_tab = rel_embed @ Wk_b       # (32,)

    # ---- per-core edge assignment (destination sharding) ----
    kg_core = np.minimum(h_list // EK_SH, NCORE - 1)
    ig_part_item = a_row < N_ENT
    ig_core = np.where(ig_part_item,
                       np.minimum(a_row // EI_SH_I, NCORE - 1),
                       np.minimum((a_row - N_ENT) // EI_SH_U, NCORE - 1))
    ig_local = np.where(ig_part_item,
                        a_row - (np.minimum(a_row // EI_SH_I, NCORE - 1)) * EI_SH_I,
                        IG_UOFF + (a_row - N_ENT) - (np.minimum((a_row - N_ENT) // EI_SH_U, NCORE - 1)) * EI_SH_U)
    kg_local = h_list - kg_core * EK_SH

    # ---- rounds per core (shared across both layers: same index data) ----
    kg_rounds = [_rounds(kg_local[kg_core == c]) for c in range(NCORE)]
    ig_rounds = [_rounds(ig_local[ig_core == c]) for c in range(NCORE)]
    kg_eids = [np.flatnonzero(kg_core == c) for c in range(NCORE)]
    ig_eids = [np.flatnonzero(ig_core == c) for c in range(NCORE)]

    nrk = max(len(r) for r in kg_rounds)
    nri = max(len(r) for r in ig_rounds)
    kg_sizes = [max((len(kg_rounds[c][j]) if j < len(kg_rounds[c]) else 1) for c in range(NCORE)) for j in range(nrk)]
    ig_sizes = [max((len(ig_rounds[c][j]) if j < len(ig_rounds[c]) else 1) for c in range(NCORE)) for j in range(nri)]
    kg_sizes = [((s + P - 1) // P) * P for s in kg_sizes]
    ig_sizes = [((s + P - 1) // P) * P for s in ig_sizes]
    ekp = sum(kg_sizes)
    eip = sum(ig_sizes)

    # per-core slot-ordered edge arrays
    kg_slots = []   # (t_idx, r_idx, h_local, valid) per core in slot order
    ig_slots = []
    for c in range(NCORE):
        tks, rks, sks, val = [], [], [], []
        for j, cap in enumerate(kg_sizes):
            if j < len(kg_rounds[c]):
                e = kg_eids[c][kg_rounds[c][j]]
            else:
                e = np.empty(0, np.int64)
            pad = cap - len(e)
            tks.append(np.r_[t_list[e], np.zeros(pad, np.int64)])
            rks.append(np.r_[r_list[e], np.zeros(pad, np.int64)])
            sks.append(np.r_[kg_local[e], np.full(pad, TRASH_K, np.int64)])
            val.append(np.r_[np.ones(len(e), bool), np.zeros(pad, bool)])
        kg_slots.append((np.concatenate(tks), np.concatenate(rks),
                         np.concatenate(sks), np.concatenate(val)))
        cks, vks, sks2 = [], [], []
        for j, cap in enumerate(ig_sizes):
            if j < len(ig_rounds[c]):
                e = ig_eids[c][ig_rounds[c][j]]
            else:
                e = np.empty(0, np.int64)
            pad = cap - len(e)
            cks.append(np.r_[a_col[e], np.zeros(pad, np.int64)])
            vks.append(np.r_[a_vals[e], np.zeros(pad, f)])
            sks2.append(np.r_[ig_local[e], np.full(pad, TRASH_I, np.int64)])
        ig_slots.append((np.concatenate(cks), np.concatenate(vks),
                         np.concatenate(sks2)))

    nc = _build_graph(ekp, eip, kg_sizes, ig_sizes)

    def slotview(x):
        # slot i lives at [i%128, i//128] on device
        return np.ascontiguousarray(x.reshape(-1, P).T)

    def run_layer(e_ent_curr, ig_in):
        q2_all = e_ent_curr @ B_tab.T + c_tab[None, :]   # (N_ENT, 32)
        in_maps = []
        for c in range(NCORE):
            tk, rk, sk, val = kg_slots[c]
            T = e_ent_curr[tk]
            A = A_tab[rk] * val[:, None]
            q = np.where(val, q2_all[np.minimum(sk + c * EK_SH, N_ENT - 1), rk], -1e4).astype(f)
            ck, vv, si = ig_slots[c]
            R = ig_in[ck]
            in_maps.append(dict(
                kg_T=T.astype(f), kg_A=A.astype(f), kg_q=slotview(q),
                kg_s=_wrap16(sk.astype(np.int16), ekp),
                ig_R=R.astype(f), ig_v=slotview(vv.astype(f)),
                ig_s=_wrap16(si.astype(np.int16), eip),
            ))
        res = run_bass_kernel_spmd(nc, in_maps, list(range(NCORE)))
        if res.exec_time_ns:
            LAST_EXEC_NS.append(res.exec_time_ns)
        kg_full = np.empty((N_ENT, D), f)
        ig_full = np.empty((N_TOT, D), f)
        for c in range(NCORE):
            ak = np.asarray(res.results[c]["acc_k"], f)
            ai = np.asarray(res.results[c]["acc_i"], f)
            num = ak[:EK_SH, :D]
            den = ak[:EK_SH, D : D + 1]
            kg_full[c * EK_SH : (c + 1) * EK_SH] = num / (den + 1e-20)
            ig_full[c * EI_SH_I : (c + 1) * EI_SH_I] = ai[:EI_SH_I, :]
            ig_full[N_ENT + c * EI_SH_U : N_ENT + (c + 1) * EI_SH_U] = ai[IG_UOFF : IG_UOFF + EI_SH_U, :]
        return kg_full, ig_full

    e_ent = all_embed[:N_ENT]
    e_usr = all_embed[N_ENT:]
    e_ent_curr, e_dual, e_users = e_ent, e_ent, e_usr
    item_sum = e_ent.copy()
    user_sum = e_usr.copy()
    for _ in range(2):
        kg, ig = run_layer(e_ent_curr, np.concatenate([e_dual, e_users], 0))
        collab = ig[:N_ENT]
        users_new = ig[N_ENT:]
        gate = 1.0 / (1.0 + np.exp(-(kg @ Wa_w.T + collab @ Wb_w.T)))
        e_dual = gate * kg + (1.0 - gate) * collab
        item_sum += collab
        user_sum += users_new
        e_users = users_new
        e_ent_curr = kg

    all_final = np.concatenate([item_sum, user_sum], 0)
    return (all_final[user_ids] @ all_final[item_ids].T).astype(f)
